# revision 58
# baseline (speedup 1.0000x reference)
"""Trainium2 Bass kernel for a quantized ResNet BasicBlock.

Reference computation (per reference.py):
    out = act_quant(x); out = conv3x3(out, weight_quant(w1)); out = BN(out, g1, b1)
    out = act_quant(out); out = conv3x3(out, weight_quant(w2)); out = BN(out, g2, b2)
    return out + x
with act_quant(x) = round(clip(x,0,1)*15)/15 (4-bit), weight_quant symmetric 4-bit
per-tensor (levels -7..7, scale alpha/7, alpha = max|w|), BN in training mode
(batch stats over (N,H,W)).

Strategy (8 NeuronCores, data-parallel over batch, sync-BN via AllReduce):
  * Quantized activations are integers 0..15, weights integers -7..7 - both
    exact in fp8e4m3, and fp32 PSUM accumulation never rounds (sums < 2^24),
    so each conv3x3 is an EXACT integer computation.
  * Rounding is done by writing 15x+128 to bf16 (the [128,256) binade has
    step exactly 1, RNE matches jnp.round), then clip to [128,143] and
    subtract 128 into the unbiased fp8 activation image.
  * conv3x3 over a zero-padded [C=128 partitions, 59, 64] fp8 image
    (64-wide rows give the 16B-aligned row stride fp8 DoubleRow needs):
    per 8-row output group, 3 DoubleRow pair-matmuls contract taps
    (0,dw)+(1,dw) as K=256 in one pass (rhs is an overlapping [C,2,512]
    access pattern, pair stride = one row) plus 3 normal matmuls for the
    (2,dw) taps - 6 PE instructions instead of 9, streaming full 64-wide
    rows into a [C,8,64] PSUM bank (alignment columns are zero/ignored).
  * The PSUM->SBUF copy (ACT, accum_out) emits per-channel BN sums and
    stores conv results as int16 (|conv_int| ~ 2.3k); sum-of-squares via a
    DVE scalar_tensor_tensor with accum_out.  Per-channel sum/sumsq are
    AllReduced across the 8 cores ([128,2] fp32), then BN+act_quant collapse
    into one per-channel scale/bias applied to the integer conv output.
  * x is kept resident in SBUF as bf16 for the residual (no HBM reload;
    adds <=2^-9 relative rounding on the identity term, well inside the
    2e-2 gate), and the final BN2+residual is streamed per 28-row chunk
    straight to the output DMA so the post-AllReduce tail is store-bound.
"""

import os
import sys

for _p in ("/opt/trn_rl_repo", "/root/.axon_site/_ro/trn_rl_repo"):
    if os.path.isdir(_p) and _p not in sys.path:
        sys.path.insert(0, _p)

import numpy as np
import ml_dtypes

import concourse.bass as bass  # noqa: F401  (registers types)
import concourse.tile as tile
from concourse import bacc, mybir
from concourse import bass_utils

F32 = mybir.dt.float32
BF16 = mybir.dt.bfloat16
I16 = mybir.dt.int16
I32 = mybir.dt.int32
F8 = mybir.dt.float8e4
ACTF = mybir.ActivationFunctionType
ALU = mybir.AluOpType
AX = mybir.AxisListType

C = 128
H = W = 56
HP = 59               # padded rows (+1 spare zero row for stream overshoot)
WP = 64               # padded cols (16B-aligned rows for fp8 DoubleRow)
GR = 8                # output rows per PSUM group
NG = H // GR          # 7 groups per image
NCORES = 8

# cvec column indices (all [C] fp32, host-computed)
CV_INVM, CV_K1, CV_K2, CV_SM1, CV_SM2, CV_CA1, CV_CB1, CV_CA2, CV_CB2, \
    CV_EPS, CV_C15, CV_C128, CV_NCOLS = range(13)


def _bn_coefs(nc, pool, S, SS, cvcol, ph):
    """[C,1] coef math from global integer-unit sum S / sumsq SS to the fused
    scale/bias for this BN + following op.

    ph=1: (uscale, ubias): u = conv_int*uscale + ubias = 15*BN(y) + 128.
    ph=2: (fscale, fbias): out = conv_int*fscale + fbias = BN(y2).
    """
    idx = [0]

    def mk():
        idx[0] += 1
        return pool.tile([C, 1], F32, tag=f"bc{ph}_{idx[0]}", name=f"bc{ph}_{idx[0]}")

    kcol = CV_K1 if ph == 1 else CV_K2
    smcol = CV_SM1 if ph == 1 else CV_SM2
    acol = CV_CA1 if ph == 1 else CV_CA2
    bcol = CV_CB1 if ph == 1 else CV_CB2

    # critical path: ms -> ms2 -> vpe -> sqrt -> recip -> newton -> scale -> bias;
    # mean / sv / us0 hang off it in parallel (Pool).
    ms = mk()   # mean in real units, = S*(s/m)
    nc.vector.tensor_scalar(ms[:], S, cvcol(smcol), None, op0=ALU.mult)
    sv = mk()   # SS*s^2/m + eps  (on Pool, parallel)
    nc.gpsimd.tensor_scalar(sv[:], SS, cvcol(kcol), cvcol(CV_EPS),
                            op0=ALU.mult, op1=ALU.add)
    mean = mk()  # mean in integer units (only needed for the bias at the end)
    nc.gpsimd.tensor_scalar(mean[:], S, cvcol(CV_INVM), None, op0=ALU.mult)
    ms2 = mk()
    nc.vector.tensor_tensor(out=ms2[:], in0=ms[:], in1=ms[:], op=ALU.mult)
    vpe = mk()  # var_real + eps
    nc.vector.tensor_tensor(out=vpe[:], in0=sv[:], in1=ms2[:], op=ALU.subtract)
    # rsqrt without the scalar engine (avoids Sqrt<->Identity act-table
    # reloads): fast-inverse-sqrt seed j = MAGIC - (bits(v) >> 1) computed as
    # MAGIC - 0.5*bits(v) in the fp32 datapath (<=64-ulp bit error on a 3.4%
    # seed), then two Newton steps r <- r*(1.5 - 0.5*v*r^2) for ~5e-6 rel.
    seed = pool.tile([C, 1], I32, tag=f"bseed{ph}", name=f"bseed{ph}")
    nc.vector.tensor_scalar(seed[:], vpe[:].bitcast(I32), -0.5, 1597463007.0,
                            op0=ALU.mult, op1=ALU.add)
    r0 = seed[:].bitcast(F32)
    t1 = mk()
    nc.vector.tensor_tensor(out=t1[:], in0=r0, in1=r0, op=ALU.mult)
    nc.vector.tensor_tensor(out=t1[:], in0=t1[:], in1=vpe[:], op=ALU.mult)
    nc.vector.tensor_scalar(t1[:], t1[:], -0.5, 1.5, op0=ALU.mult, op1=ALU.add)
    r1 = mk()
    nc.vector.tensor_tensor(out=r1[:], in0=r0, in1=t1[:], op=ALU.mult)
    # us0 = CA*r1 on Pool, parallel with the second Newton chain
    us0 = mk()
    nc.gpsimd.tensor_tensor(out=us0[:], in0=cvcol(acol), in1=r1[:], op=ALU.mult)
    t2 = mk()
    nc.vector.tensor_tensor(out=t2[:], in0=r1[:], in1=r1[:], op=ALU.mult)
    nc.vector.tensor_tensor(out=t2[:], in0=t2[:], in1=vpe[:], op=ALU.mult)
    nc.vector.tensor_scalar(t2[:], t2[:], -0.5, 1.5, op0=ALU.mult, op1=ALU.add)
    scale = mk()
    nc.vector.tensor_tensor(out=scale[:], in0=us0[:], in1=t2[:], op=ALU.mult)
    mb = mk()
    nc.vector.tensor_tensor(out=mb[:], in0=scale[:], in1=mean[:], op=ALU.mult)
    bias = mk()
    nc.vector.scalar_tensor_tensor(out=bias[:], in0=mb[:], scalar=-1.0,
                                   in1=cvcol(bcol), op0=ALU.mult, op1=ALU.add)
    return scale, bias


def build_program(ncores, nper, collective=True, reps=1):
    nc = bacc.Bacc("TRN2", target_bir_lowering=False, debug=False, num_devices=ncores)

    x_in = nc.dram_tensor("x", [nper, C, H, W], F32, kind="ExternalInput")
    w1_in = nc.dram_tensor("w1s", [C, 9, C], F8, kind="ExternalInput")
    w2_in = nc.dram_tensor("w2s", [C, 9, C], F8, kind="ExternalInput")
    cv_in = nc.dram_tensor("cvec", [C, CV_NCOLS], F32, kind="ExternalInput")
    out_d = nc.dram_tensor("out", [nper, C, H, W], F32, kind="ExternalOutput")

    with tile.TileContext(nc) as tc:
        with tc.tile_pool(name="const", bufs=1) as cpool, \
             tc.tile_pool(name="apad", bufs=nper) as apool, \
             tc.tile_pool(name="cint", bufs=nper) as ipool, \
             tc.tile_pool(name="xres", bufs=nper) as xrespool, \
             tc.tile_pool(name="xin", bufs=3) as xpool, \
             tc.tile_pool(name="tr", bufs=3) as trpool, \
             tc.tile_pool(name="sq", bufs=2) as sqpool, \
             tc.tile_pool(name="fin", bufs=5) as fpool, \
             tc.tile_pool(name="stat", bufs=1) as spool, \
             tc.tile_pool(name="psum", bufs=1, space="PSUM") as ppool, \
             tc.tile_pool(name="dram", bufs=1, space="DRAM") as dpool:

            tw1 = cpool.tile([C, 9, C], F8, tag="w1")
            tw2 = cpool.tile([C, 9, C], F8, tag="w2")
            tcv = cpool.tile([C, CV_NCOLS], F32, tag="cv")
            # cvec on SP first (x loads queue right behind it); weight DMAs
            # are emitted after the first image's prep, via Pool's SWDGE, so
            # the first x chunk wins the DMA device
            nc.sync.dma_start(tcv[:], cv_in.ap())

            def cvcol(j):
                return tcv[:, j:j + 1]

            # dep-free Identity op at t=0 soaks up the one-time act-table load
            warm = cpool.tile([C, 1], F32, tag="warm")
            nc.scalar.activation(warm[:], warm[:], ACTF.Identity, bias=0.0, scale=1.0)

            # slab 0 = padded activation image; slab 1 = the same image
            # shifted left one column (built by an SBUF->SBUF DMA before
            # conv2) so taps (2,0)+(2,1) pair as one DoubleRow matmul with
            # pair stride HP*WP (16B-aligned)
            apad = [apool.tile([C, 2, HP, WP], F8, tag="apad", name=f"apad{i}") for i in range(nper)]
            cint = [ipool.tile([C, H, W], I16, tag="cint", name=f"cint{i}") for i in range(nper)]
            xres = [xrespool.tile([C, H, W], BF16, tag="xres", name=f"xres{i}") for i in range(nper)]

            rep_ctx = tc.For_i(0, reps, 1) if reps > 1 else None
            if rep_ctx is not None:
                rep_ctx.__enter__()

            GA = 4                               # groups in PSUM tile A
            GB = NG - GA                         # groups in PSUM tile B
            npart = nper * 2 + 1                 # +1: last image's split B
            s1p = spool.tile([C, npart], F32, tag="s1p")
            ss1p = spool.tile([C, npart], F32, tag="ss1p")
            s2p = spool.tile([C, npart], F32, tag="s1p", name="s2p")
            ss2p = spool.tile([C, npart], F32, tag="ss1p", name="ss2p")
            psA = ppool.tile([C, GA * GR, WP], F32, tag="psA", bufs=1)
            psB = ppool.tile([C, GB * GR, WP], F32, tag="psB", bufs=1)

            def conv(i, tw, sp, ssp, pair_row2=False, sq_a_act=False):
                """conv3x3 of apad[i], group-major: per 8-row output group,
                3 DoubleRow pair-matmuls (taps (0,dw)+(1,dw), K=256) + the
                dh=2 taps (3 singles, or with pair_row2 one cross-slab DR
                for (2,0)+(2,1) plus one single) into one PSUM bank; groups
                0-3 fill the 4-bank tile A, groups 4-6 the 3-bank tile B,
                and each tile's single PSUM->SBUF copy (ACT, accum_out=BN
                sum) drains while the other tile's matmuls stream."""
                flat = apad[i].rearrange("c s h w -> c (s h w)")
                NFLAT = GR * WP

                def emit_group(g, psview):
                    out = psview.rearrange("c a b -> c (a b)")
                    for p in range(3):           # DR pairs, dw = p
                        base = g * GR * WP + p
                        rhs = flat[:, base:base + NFLAT]
                        rhs.ap.insert(1, [WP, 2])        # [C, 2, 512]
                        nc.tensor.matmul(out, tw[:, 2 * p:2 * p + 2, :], rhs,
                                         start=(p == 0), stop=False,
                                         perf_mode=mybir.MatmulPerfMode.DoubleRow)
                    if pair_row2:
                        # (2,0) in slab 0 pairs with (2,1) = slab 1 @ (2,0)
                        base = (g * GR + 2) * WP
                        rhs = flat[:, base:base + NFLAT]
                        rhs.ap.insert(1, [HP * WP, 2])   # cross-slab pair
                        nc.tensor.matmul(out, tw[:, 6:8, :], rhs,
                                         start=False, stop=False,
                                         perf_mode=mybir.MatmulPerfMode.DoubleRow)
                        base = (g * GR + 2) * WP + 2
                        rhs = flat[:, base:base + NFLAT]
                        nc.tensor.matmul(out, tw[:, 8, :], rhs,
                                         start=False, stop=True)
                    else:
                        for s in range(3):       # singles, dh = 2, dw = s
                            base = (g * GR + 2) * WP + s
                            rhs = flat[:, base:base + NFLAT]
                            nc.tensor.matmul(out, tw[:, 6 + s, :], rhs,
                                             start=False, stop=(s == 2))

                def copy_chunk(k, r0, pt, sq_act=False):
                    rows = pt.shape[1]
                    dst = cint[i][:, r0:r0 + rows, :]
                    nc.scalar.activation(dst, pt[:, :, 0:W], ACTF.Identity,
                                         bias=0.0, scale=1.0,
                                         accum_out=sp[:, k:k + 1])
                    sq = sqpool.tile([C, GA * GR, W], BF16, tag="sq")
                    if sq_act:
                        # ACT Square (1-ULP family, same act-table set as
                        # Identity) frees DVE in the paired conv2 regime
                        nc.scalar.activation(sq[:, 0:rows, :], dst, ACTF.Square,
                                             bias=0.0, scale=1.0,
                                             accum_out=ssp[:, k:k + 1])
                    else:
                        nc.vector.scalar_tensor_tensor(
                            out=sq[:, 0:rows, :], in0=dst, scalar=1.0, in1=dst,
                            op0=ALU.mult, op1=ALU.mult,
                            accum_out=ssp[:, k:k + 1])

                for g in range(GA):
                    emit_group(g, psA[:, g * GR:(g + 1) * GR, :])
                copy_chunk(2 * i, 0, psA[:], sq_act=sq_a_act)
                if i == nper - 1:
                    # last image: drain PSUM B in two pieces so the final
                    # copy+sumsq feeding the BN stats reduce is small
                    emit_group(GA, psB[:, 0:GR, :])
                    emit_group(GA + 1, psB[:, GR:2 * GR, :])
                    copy_chunk(2 * i + 1, GA * GR, psB[:, 0:2 * GR, :])
                    emit_group(GA + 2, psB[:, 2 * GR:3 * GR, :])
                    copy_chunk(2 * nper, (GA + 2) * GR, psB[:, 2 * GR:3 * GR, :])
                else:
                    for g in range(GB):
                        emit_group(GA + g, psB[:, g * GR:(g + 1) * GR, :])
                    copy_chunk(2 * i + 1, GA * GR, psB[:])

            def stats_allreduce(sp, ssp, tag):
                st = spool.tile([C, 2], F32, tag=f"st{tag}")
                nc.vector.tensor_reduce(out=st[:, 0:1], in_=sp[:], axis=AX.X, op=ALU.add)
                nc.vector.tensor_reduce(out=st[:, 1:2], in_=ssp[:], axis=AX.X, op=ALU.add)
                if not collective:
                    return st
                din = dpool.tile([C, 2], F32, tag=f"din{tag}")
                dout = dpool.tile([C, 2], F32, tag=f"dout{tag}")
                nc.gpsimd.dma_start(din[:], st[:])
                nc.gpsimd.collective_compute(
                    "AllReduce", ALU.add,
                    replica_groups=[list(range(ncores))],
                    ins=[din.opt()], outs=[dout.opt()])
                gst = spool.tile([C, 2], F32, tag=f"gst{tag}")
                nc.gpsimd.dma_start(gst[:], dout[:])
                return gst

            # ---------------- stage A + conv1 ----------------
            with nc.named_scope("conv1"):
                HH = H // 4

                def phase1_prep(i):
                    # zero the conv border (incl. alignment cols); on DVE, so
                    # the dep-free memsets never crowd out the clamps
                    nc.vector.memset(apad[i][:, 0, 0, :], 0)
                    nc.vector.memset(apad[i][:, 0, 57:HP, :], 0)
                    nc.vector.memset(apad[i][:, 0, 1:57, 0:1], 0)
                    nc.vector.memset(apad[i][:, 0, 1:57, 57:WP], 0)
                    # shifted slab's last column never receives DMA data
                    nc.vector.memset(apad[i][:, 1, 2:58, 63:WP], 0)
                    p1ch = [(h0, HH) for h0 in range(0, H, HH)]
                    if i == 0:
                        # small leading chunks shorten the cold-start chain
                        # into the first matmul
                        p1ch = [(0, 7), (7, 7)] + p1ch[1:]
                    for h0, hh in p1ch:
                        xt = xpool.tile([C, HH, W], F32, tag="x")
                        nc.sync.dma_start(xt[:, 0:hh, :],
                                          x_in.ap()[i][:, h0:h0 + hh, :])
                        tat = trpool.tile([C, 16, W], BF16, tag="tr", name="tat")
                        ta = tat[:, 0:hh, :]
                        # u = 15x+128 -> bf16 write rounds to integer grid
                        # (RNE); same ACT Identity scale/bias pattern phase2
                        # uses, so numerics match the DVE path
                        nc.scalar.activation(ta, xt[:, 0:hh, :], ACTF.Identity,
                                             bias=cvcol(CV_C128), scale=cvcol(CV_C15))
                        nc.vector.tensor_scalar(ta, ta, 143.0, 128.0,
                                                op0=ALU.min, op1=ALU.max)
                        intr = apad[i][:, 0, 1 + h0:1 + h0 + hh, 1:W + 1]
                        nc.vector.tensor_scalar(intr, ta, 128.0, None,
                                                op0=ALU.subtract)
                        # residual copy: keep x resident as bf16 (Pool)
                        nc.gpsimd.tensor_scalar(xres[i][:, h0:h0 + hh, :],
                                                xt[:, 0:hh, :],
                                                0.0, None, op0=ALU.add)

                # prep-ahead of 2 keeps each engine's in-order stream from
                # head-of-line-blocking the next image's prep behind this
                # image's PSUM copies
                phase1_prep(0)
                nc.gpsimd.dma_start(tw1[:], w1_in.ap())
                nc.gpsimd.dma_start(tw2[:], w2_in.ap())
                phase1_prep(1)
                for i in range(nper):
                    conv(i, tw1, s1p, ss1p)
                    if i + 2 < nper:
                        phase1_prep(i + 2)

            # ---------------- BN1 sync + coefs ----------------
            with nc.named_scope("bn1"):
                gst1 = stats_allreduce(s1p, ss1p, 1)
                uscale, ubias = _bn_coefs(nc, spool, gst1[:, 0:1], gst1[:, 1:2], cvcol, 1)

            # ---------------- phase2 (act_quant of BN1) + conv2 ----------------
            with nc.named_scope("conv2"):
                # first chunk covers rows 0..11 so group-0 matmuls (rows 0..9)
                # wait on one small chunk only
                P2CH = [(0, 12), (12, 16), (28, 16), (44, 12)]

                def phase2_prep(i):
                    for r0, rows in P2CH:
                        src = cint[i][:, r0:r0 + rows, :]
                        dsta = apad[i][:, 0, 1 + r0:1 + r0 + rows, 1:W + 1]
                        tr = trpool.tile([C, 16, W], BF16, tag="tr")
                        trv = tr[:, 0:rows, :]
                        nc.gpsimd.tensor_scalar(trv, src, uscale[:], ubias[:],
                                                op0=ALU.mult, op1=ALU.add)
                        nc.vector.tensor_scalar(trv, trv, 143.0, 128.0,
                                                op0=ALU.min, op1=ALU.max)
                        nc.vector.tensor_scalar(dsta, trv, 128.0, None,
                                                op0=ALU.subtract)
                    # build the shifted slab for the row-2 DoubleRow pair:
                    # slab1[r, 0:63] = slab0[r, 1:64] for the rows taps read
                    # (SBUF->SBUF DMA; the DMA engines are idle during conv2);
                    # two halves so the early groups' rows land sooner
                    if i >= 2:
                        nc.sync.dma_start(apad[i][:, 1, 2:34, 0:63],
                                          apad[i][:, 0, 2:34, 1:WP])
                        nc.sync.dma_start(apad[i][:, 1, 34:58, 0:63],
                                          apad[i][:, 0, 34:58, 1:WP])

                phase2_prep(0)
                phase2_prep(1)
                phase2_prep(2)
                for i in range(nper):
                    # images 0/1 keep the 3-single path: their shifted slab
                    # (gated by the BN1 coefs) isn't ready yet when PE is
                    conv(i, tw2, s2p, ss2p, pair_row2=(i >= 2),
                         sq_a_act=(i < nper - 1))
                    if i + 3 < nper:
                        phase2_prep(i + 3)

            # ---------------- BN2 sync + coefs ----------------
            with nc.named_scope("bn2"):
                gst2 = stats_allreduce(s2p, ss2p, 2)
                fscale, fbias = _bn_coefs(nc, spool, gst2[:, 0:1], gst2[:, 1:2], cvcol, 2)

            # ---------------- finalize: BN2 + residual, streamed stores ----------------
            with nc.named_scope("finalize"):
                CH = 14              # finalize chunk rows (divides H)
                NCH = H // CH
                for i in range(nper):
                    chunks = [(cidx * CH, CH) for cidx in range(NCH)]
                    if i == 0:
                        chunks = [(0, 7), (7, 7)] + chunks[1:]
                    for r0, rows in chunks:
                        src = cint[i][:, r0:r0 + rows, :]
                        t2f = fpool.tile([C, CH, W], F32, tag="fin", name="t2f")
                        t2 = t2f[:, 0:rows, :]
                        nc.scalar.activation(t2, src, ACTF.Identity,
                                             bias=fbias[:], scale=fscale[:])
                        # residual add in place; deep fpool keeps the store
                        # pipeline DMA-bound instead of buffer-reuse-bound
                        nc.vector.tensor_tensor(out=t2, in0=t2,
                                                in1=xres[i][:, r0:r0 + rows, :], op=ALU.add)
                        nc.sync.dma_start(out_d.ap()[i][:, r0:r0 + rows, :], t2)

            if rep_ctx is not None:
                rep_ctx.__exit__(None, None, None)

    nc.compile()
    return nc


_PROG_CACHE = {}


def _get_program(ncores, nper):
    key = (ncores, nper)
    if key not in _PROG_CACHE:
        _PROG_CACHE[key] = build_program(ncores, nper)
    return _PROG_CACHE[key]


def make_inputs(x, w1, w2, gamma1, beta1, gamma2, beta2, ncores=NCORES):
    """Host-side prep: shard x, quantize weights, build cvec."""
    x = np.asarray(x, dtype=np.float32)
    n = x.shape[0]
    nper = n // ncores
    assert nper * ncores == n

    def wq(w):
        w = np.asarray(w, dtype=np.float32)
        alpha = np.float32(np.abs(w).max()) + np.float32(1e-12)
        q = np.round(np.clip(w / alpha, -1.0, 1.0) * np.float32(7.0))
        return q.astype(np.float32), np.float32(alpha)

    q1, a1 = wq(w1)
    q2, a2 = wq(w2)
    # [co, ci, kh, kw] -> [ci, j, co], j ordered as DoubleRow pairs
    # [(0,dw),(1,dw)] for dw=0..2 then singles [(2,dw)]
    f8np = mybir.dt.np(F8)
    order = [(0, 0), (1, 0), (0, 1), (1, 1), (0, 2), (1, 2), (2, 0), (2, 1), (2, 2)]

    def pack(q):
        t = q.transpose(1, 2, 3, 0)
        return np.ascontiguousarray(
            np.stack([t[:, kh, kw, :] for kh, kw in order], axis=1)).astype(f8np)

    w1s = pack(q1)
    w2s = pack(q2)
    s1 = np.float32(a1 / np.float32(105.0))
    s2 = np.float32(a2 / np.float32(105.0))
    m = np.float32(n * H * W)

    g1 = np.asarray(gamma1, dtype=np.float32)
    g2 = np.asarray(gamma2, dtype=np.float32)
    b1 = np.asarray(beta1, dtype=np.float32)
    b2 = np.asarray(beta2, dtype=np.float32)

    cvec = np.zeros((C, CV_NCOLS), dtype=np.float32)
    cvec[:, CV_INVM] = np.float32(1.0) / m
    cvec[:, CV_K1] = s1 * s1 / m
    cvec[:, CV_K2] = s2 * s2 / m
    cvec[:, CV_SM1] = s1 / m
    cvec[:, CV_SM2] = s2 / m
    cvec[:, CV_CA1] = np.float32(15.0) * s1 * g1
    cvec[:, CV_CB1] = np.float32(15.0) * b1 + np.float32(128.0)
    cvec[:, CV_CA2] = s2 * g2
    cvec[:, CV_CB2] = b2
    cvec[:, CV_EPS] = 1e-5
    cvec[:, CV_C15] = 15.0
    cvec[:, CV_C128] = 128.0

    in_maps = []
    for c in range(ncores):
        in_maps.append({
            "x": np.ascontiguousarray(x[c * nper:(c + 1) * nper]),
            "w1s": w1s, "w2s": w2s, "cvec": cvec,
        })
    return in_maps, nper


def run(x, w1, w2, gamma1, beta1, gamma2, beta2, trace=False):
    in_maps, nper = make_inputs(x, w1, w2, gamma1, beta1, gamma2, beta2)
    nc = _get_program(NCORES, nper)
    res = bass_utils.run_bass_kernel_spmd(
        nc, in_maps, core_ids=list(range(NCORES)), trace=trace)
    out = np.concatenate([r["out"] for r in res.results], axis=0)
    return out, res


def kernel(x, w1, w2, gamma1, beta1, gamma2, beta2):
    out, _ = run(x, w1, w2, gamma1, beta1, gamma2, beta2)
    return out


# revision 59
# speedup vs baseline: 1.0049x; 1.0049x over previous
"""Trainium2 Bass kernel for a quantized ResNet BasicBlock.

Reference computation (per reference.py):
    out = act_quant(x); out = conv3x3(out, weight_quant(w1)); out = BN(out, g1, b1)
    out = act_quant(out); out = conv3x3(out, weight_quant(w2)); out = BN(out, g2, b2)
    return out + x
with act_quant(x) = round(clip(x,0,1)*15)/15 (4-bit), weight_quant symmetric 4-bit
per-tensor (levels -7..7, scale alpha/7, alpha = max|w|), BN in training mode
(batch stats over (N,H,W)).

Strategy (8 NeuronCores, data-parallel over batch, sync-BN via AllReduce):
  * Quantized activations are integers 0..15, weights integers -7..7 - both
    exact in fp8e4m3, and fp32 PSUM accumulation never rounds (sums < 2^24),
    so each conv3x3 is an EXACT integer computation.
  * Rounding is done by writing 15x+128 to bf16 (the [128,256) binade has
    step exactly 1, RNE matches jnp.round), then clip to [128,143] and
    subtract 128 into the unbiased fp8 activation image.
  * conv3x3 over a zero-padded [C=128 partitions, 59, 64] fp8 image
    (64-wide rows give the 16B-aligned row stride fp8 DoubleRow needs):
    per 8-row output group, 3 DoubleRow pair-matmuls contract taps
    (0,dw)+(1,dw) as K=256 in one pass (rhs is an overlapping [C,2,512]
    access pattern, pair stride = one row) plus 3 normal matmuls for the
    (2,dw) taps - 6 PE instructions instead of 9, streaming full 64-wide
    rows into a [C,8,64] PSUM bank (alignment columns are zero/ignored).
  * The PSUM->SBUF copy (ACT, accum_out) emits per-channel BN sums and
    stores conv results as int16 (|conv_int| ~ 2.3k); sum-of-squares via a
    DVE scalar_tensor_tensor with accum_out.  Per-channel sum/sumsq are
    AllReduced across the 8 cores ([128,2] fp32), then BN+act_quant collapse
    into one per-channel scale/bias applied to the integer conv output.
  * x is kept resident in SBUF as bf16 for the residual (no HBM reload;
    adds <=2^-9 relative rounding on the identity term, well inside the
    2e-2 gate), and the final BN2+residual is streamed per 28-row chunk
    straight to the output DMA so the post-AllReduce tail is store-bound.
"""

import os
import sys

for _p in ("/opt/trn_rl_repo", "/root/.axon_site/_ro/trn_rl_repo"):
    if os.path.isdir(_p) and _p not in sys.path:
        sys.path.insert(0, _p)

import numpy as np
import ml_dtypes

import concourse.bass as bass  # noqa: F401  (registers types)
import concourse.tile as tile
from concourse import bacc, mybir
from concourse import bass_utils

F32 = mybir.dt.float32
BF16 = mybir.dt.bfloat16
I16 = mybir.dt.int16
I32 = mybir.dt.int32
F8 = mybir.dt.float8e4
ACTF = mybir.ActivationFunctionType
ALU = mybir.AluOpType
AX = mybir.AxisListType

C = 128
H = W = 56
HP = 59               # padded rows (+1 spare zero row for stream overshoot)
WP = 64               # padded cols (16B-aligned rows for fp8 DoubleRow)
GR = 8                # output rows per PSUM group
NG = H // GR          # 7 groups per image
NCORES = 8

# cvec column indices (all [C] fp32, host-computed)
CV_INVM, CV_K1, CV_K2, CV_SM1, CV_SM2, CV_CA1, CV_CB1, CV_CA2, CV_CB2, \
    CV_EPS, CV_C15, CV_C128, CV_NCOLS = range(13)


def _bn_coefs(nc, pool, S, SS, cvcol, ph):
    """[C,1] coef math from global integer-unit sum S / sumsq SS to the fused
    scale/bias for this BN + following op.

    ph=1: (uscale, ubias): u = conv_int*uscale + ubias = 15*BN(y) + 128.
    ph=2: (fscale, fbias): out = conv_int*fscale + fbias = BN(y2).
    """
    idx = [0]

    def mk():
        idx[0] += 1
        return pool.tile([C, 1], F32, tag=f"bc{ph}_{idx[0]}", name=f"bc{ph}_{idx[0]}")

    kcol = CV_K1 if ph == 1 else CV_K2
    smcol = CV_SM1 if ph == 1 else CV_SM2
    acol = CV_CA1 if ph == 1 else CV_CA2
    bcol = CV_CB1 if ph == 1 else CV_CB2

    # critical path: ms -> ms2 -> vpe -> sqrt -> recip -> newton -> scale -> bias;
    # mean / sv / us0 hang off it in parallel (Pool).
    ms = mk()   # mean in real units, = S*(s/m)
    nc.vector.tensor_scalar(ms[:], S, cvcol(smcol), None, op0=ALU.mult)
    sv = mk()   # SS*s^2/m + eps  (on Pool, parallel)
    nc.gpsimd.tensor_scalar(sv[:], SS, cvcol(kcol), cvcol(CV_EPS),
                            op0=ALU.mult, op1=ALU.add)
    mean = mk()  # mean in integer units (only needed for the bias at the end)
    nc.gpsimd.tensor_scalar(mean[:], S, cvcol(CV_INVM), None, op0=ALU.mult)
    ms2 = mk()
    nc.vector.tensor_tensor(out=ms2[:], in0=ms[:], in1=ms[:], op=ALU.mult)
    vpe = mk()  # var_real + eps
    nc.vector.tensor_tensor(out=vpe[:], in0=sv[:], in1=ms2[:], op=ALU.subtract)
    # rsqrt without the scalar engine (avoids Sqrt<->Identity act-table
    # reloads): fast-inverse-sqrt seed j = MAGIC - (bits(v) >> 1) computed as
    # MAGIC - 0.5*bits(v) in the fp32 datapath (<=64-ulp bit error on a 3.4%
    # seed), then two Newton steps r <- r*(1.5 - 0.5*v*r^2) for ~5e-6 rel.
    seed = pool.tile([C, 1], I32, tag=f"bseed{ph}", name=f"bseed{ph}")
    nc.vector.tensor_scalar(seed[:], vpe[:].bitcast(I32), -0.5, 1597463007.0,
                            op0=ALU.mult, op1=ALU.add)
    r0 = seed[:].bitcast(F32)
    t1 = mk()
    nc.vector.tensor_tensor(out=t1[:], in0=r0, in1=r0, op=ALU.mult)
    nc.vector.tensor_tensor(out=t1[:], in0=t1[:], in1=vpe[:], op=ALU.mult)
    nc.vector.tensor_scalar(t1[:], t1[:], -0.5, 1.5, op0=ALU.mult, op1=ALU.add)
    r1 = mk()
    nc.vector.tensor_tensor(out=r1[:], in0=r0, in1=t1[:], op=ALU.mult)
    # us0 = CA*r1 on Pool, parallel with the second Newton chain
    us0 = mk()
    nc.gpsimd.tensor_tensor(out=us0[:], in0=cvcol(acol), in1=r1[:], op=ALU.mult)
    t2 = mk()
    nc.vector.tensor_tensor(out=t2[:], in0=r1[:], in1=r1[:], op=ALU.mult)
    nc.vector.tensor_tensor(out=t2[:], in0=t2[:], in1=vpe[:], op=ALU.mult)
    nc.vector.tensor_scalar(t2[:], t2[:], -0.5, 1.5, op0=ALU.mult, op1=ALU.add)
    scale = mk()
    nc.vector.tensor_tensor(out=scale[:], in0=us0[:], in1=t2[:], op=ALU.mult)
    mb = mk()
    nc.vector.tensor_tensor(out=mb[:], in0=scale[:], in1=mean[:], op=ALU.mult)
    bias = mk()
    nc.vector.scalar_tensor_tensor(out=bias[:], in0=mb[:], scalar=-1.0,
                                   in1=cvcol(bcol), op0=ALU.mult, op1=ALU.add)
    return scale, bias


def build_program(ncores, nper, collective=True, reps=1):
    nc = bacc.Bacc("TRN2", target_bir_lowering=False, debug=False, num_devices=ncores)

    x_in = nc.dram_tensor("x", [nper, C, H, W], F32, kind="ExternalInput")
    w1_in = nc.dram_tensor("w1s", [C, 9, C], F8, kind="ExternalInput")
    w2_in = nc.dram_tensor("w2s", [C, 9, C], F8, kind="ExternalInput")
    cv_in = nc.dram_tensor("cvec", [C, CV_NCOLS], F32, kind="ExternalInput")
    out_d = nc.dram_tensor("out", [nper, C, H, W], F32, kind="ExternalOutput")

    with tile.TileContext(nc) as tc:
        with tc.tile_pool(name="const", bufs=1) as cpool, \
             tc.tile_pool(name="apad", bufs=nper) as apool, \
             tc.tile_pool(name="cint", bufs=nper) as ipool, \
             tc.tile_pool(name="xres", bufs=nper) as xrespool, \
             tc.tile_pool(name="xin", bufs=3) as xpool, \
             tc.tile_pool(name="tr", bufs=3) as trpool, \
             tc.tile_pool(name="sq", bufs=2) as sqpool, \
             tc.tile_pool(name="fin", bufs=5) as fpool, \
             tc.tile_pool(name="stat", bufs=1) as spool, \
             tc.tile_pool(name="psum", bufs=1, space="PSUM") as ppool, \
             tc.tile_pool(name="dram", bufs=1, space="DRAM") as dpool:

            tw1 = cpool.tile([C, 9, C], F8, tag="w1")
            tw2 = cpool.tile([C, 9, C], F8, tag="w2")
            tcv = cpool.tile([C, CV_NCOLS], F32, tag="cv")
            # cvec on SP first (x loads queue right behind it); weight DMAs
            # are emitted after the first image's prep, via Pool's SWDGE, so
            # the first x chunk wins the DMA device
            nc.sync.dma_start(tcv[:], cv_in.ap())

            def cvcol(j):
                return tcv[:, j:j + 1]

            # dep-free Identity op at t=0 soaks up the one-time act-table load
            warm = cpool.tile([C, 1], F32, tag="warm")
            nc.scalar.activation(warm[:], warm[:], ACTF.Identity, bias=0.0, scale=1.0)

            # slab 0 = padded activation image; slab 1 = the same image
            # shifted left one column (built by an SBUF->SBUF DMA before
            # conv2) so taps (2,0)+(2,1) pair as one DoubleRow matmul with
            # pair stride HP*WP (16B-aligned)
            apad = [apool.tile([C, 2, HP, WP], F8, tag="apad", name=f"apad{i}") for i in range(nper)]
            cint = [ipool.tile([C, H, W], I16, tag="cint", name=f"cint{i}") for i in range(nper)]
            xres = [xrespool.tile([C, H, W], BF16, tag="xres", name=f"xres{i}") for i in range(nper)]

            rep_ctx = tc.For_i(0, reps, 1) if reps > 1 else None
            if rep_ctx is not None:
                rep_ctx.__enter__()

            GA = 4                               # groups in PSUM tile A
            GB = NG - GA                         # groups in PSUM tile B
            npart = nper * 2 + 1                 # +1: last image's split B
            s1p = spool.tile([C, npart], F32, tag="s1p")
            ss1p = spool.tile([C, npart], F32, tag="ss1p")
            s2p = spool.tile([C, npart], F32, tag="s1p", name="s2p")
            ss2p = spool.tile([C, npart], F32, tag="ss1p", name="ss2p")
            psA = ppool.tile([C, GA * GR, WP], F32, tag="psA", bufs=1)
            psB = ppool.tile([C, GB * GR, WP], F32, tag="psB", bufs=1)

            def conv(i, tw, sp, ssp, pair_row2=False, sq_a_act=False):
                """conv3x3 of apad[i], group-major: per 8-row output group,
                3 DoubleRow pair-matmuls (taps (0,dw)+(1,dw), K=256) + the
                dh=2 taps (3 singles, or with pair_row2 one cross-slab DR
                for (2,0)+(2,1) plus one single) into one PSUM bank; groups
                0-3 fill the 4-bank tile A, groups 4-6 the 3-bank tile B,
                and each tile's single PSUM->SBUF copy (ACT, accum_out=BN
                sum) drains while the other tile's matmuls stream."""
                flat = apad[i].rearrange("c s h w -> c (s h w)")
                NFLAT = GR * WP

                def emit_group(g, psview):
                    out = psview.rearrange("c a b -> c (a b)")
                    for p in range(3):           # DR pairs, dw = p
                        base = g * GR * WP + p
                        rhs = flat[:, base:base + NFLAT]
                        rhs.ap.insert(1, [WP, 2])        # [C, 2, 512]
                        nc.tensor.matmul(out, tw[:, 2 * p:2 * p + 2, :], rhs,
                                         start=(p == 0), stop=False,
                                         perf_mode=mybir.MatmulPerfMode.DoubleRow)
                    if pair_row2:
                        # (2,0) in slab 0 pairs with (2,1) = slab 1 @ (2,0)
                        base = (g * GR + 2) * WP
                        rhs = flat[:, base:base + NFLAT]
                        rhs.ap.insert(1, [HP * WP, 2])   # cross-slab pair
                        nc.tensor.matmul(out, tw[:, 6:8, :], rhs,
                                         start=False, stop=False,
                                         perf_mode=mybir.MatmulPerfMode.DoubleRow)
                        base = (g * GR + 2) * WP + 2
                        rhs = flat[:, base:base + NFLAT]
                        nc.tensor.matmul(out, tw[:, 8, :], rhs,
                                         start=False, stop=True)
                    else:
                        for s in range(3):       # singles, dh = 2, dw = s
                            base = (g * GR + 2) * WP + s
                            rhs = flat[:, base:base + NFLAT]
                            nc.tensor.matmul(out, tw[:, 6 + s, :], rhs,
                                             start=False, stop=(s == 2))

                def copy_chunk(k, r0, pt, sq_act=False):
                    rows = pt.shape[1]
                    dst = cint[i][:, r0:r0 + rows, :]
                    nc.scalar.activation(dst, pt[:, :, 0:W], ACTF.Identity,
                                         bias=0.0, scale=1.0,
                                         accum_out=sp[:, k:k + 1])
                    sq = sqpool.tile([C, GA * GR, W], BF16, tag="sq")
                    if sq_act:
                        # ACT Square (1-ULP family, same act-table set as
                        # Identity) frees DVE in the paired conv2 regime
                        nc.scalar.activation(sq[:, 0:rows, :], dst, ACTF.Square,
                                             bias=0.0, scale=1.0,
                                             accum_out=ssp[:, k:k + 1])
                    else:
                        nc.vector.scalar_tensor_tensor(
                            out=sq[:, 0:rows, :], in0=dst, scalar=1.0, in1=dst,
                            op0=ALU.mult, op1=ALU.mult,
                            accum_out=ssp[:, k:k + 1])

                for g in range(GA):
                    emit_group(g, psA[:, g * GR:(g + 1) * GR, :])
                copy_chunk(2 * i, 0, psA[:], sq_act=sq_a_act)
                if i == nper - 1:
                    # last image: drain PSUM B in two pieces so the final
                    # copy+sumsq feeding the BN stats reduce is small
                    emit_group(GA, psB[:, 0:GR, :])
                    emit_group(GA + 1, psB[:, GR:2 * GR, :])
                    copy_chunk(2 * i + 1, GA * GR, psB[:, 0:2 * GR, :])
                    emit_group(GA + 2, psB[:, 2 * GR:3 * GR, :])
                    copy_chunk(2 * nper, (GA + 2) * GR, psB[:, 2 * GR:3 * GR, :])
                else:
                    for g in range(GB):
                        emit_group(GA + g, psB[:, g * GR:(g + 1) * GR, :])
                    copy_chunk(2 * i + 1, GA * GR, psB[:])

            def stats_allreduce(sp, ssp, tag):
                st = spool.tile([C, 2], F32, tag=f"st{tag}")
                nc.vector.tensor_reduce(out=st[:, 0:1], in_=sp[:], axis=AX.X, op=ALU.add)
                nc.vector.tensor_reduce(out=st[:, 1:2], in_=ssp[:], axis=AX.X, op=ALU.add)
                if not collective:
                    return st
                din = dpool.tile([C, 2], F32, tag=f"din{tag}")
                dout = dpool.tile([C, 2], F32, tag=f"dout{tag}")
                nc.gpsimd.dma_start(din[:], st[:])
                nc.gpsimd.collective_compute(
                    "AllReduce", ALU.add,
                    replica_groups=[list(range(ncores))],
                    ins=[din.opt()], outs=[dout.opt()])
                gst = spool.tile([C, 2], F32, tag=f"gst{tag}")
                nc.gpsimd.dma_start(gst[:], dout[:])
                return gst

            # ---------------- stage A + conv1 ----------------
            with nc.named_scope("conv1"):
                HH = H // 4

                def phase1_prep(i):
                    # zero the conv border (incl. alignment cols); on DVE, so
                    # the dep-free memsets never crowd out the clamps
                    nc.vector.memset(apad[i][:, 0, 0, :], 0)
                    nc.vector.memset(apad[i][:, 0, 57:HP, :], 0)
                    nc.vector.memset(apad[i][:, 0, 1:57, 0:1], 0)
                    nc.vector.memset(apad[i][:, 0, 1:57, 57:WP], 0)
                    # shifted slab's last column never receives DMA data
                    nc.vector.memset(apad[i][:, 1, 2:58, 63:WP], 0)
                    for h0 in range(0, H, HH):
                        xt = xpool.tile([C, HH, W], F32, tag="x")
                        nc.sync.dma_start(xt[:], x_in.ap()[i][:, h0:h0 + HH, :])
                        tat = trpool.tile([C, 16, W], BF16, tag="tr", name="tat")
                        ta = tat[:, 0:HH, :]
                        # u = 15x+128 -> bf16 write rounds to integer grid
                        # (RNE); same ACT Identity scale/bias pattern phase2
                        # uses, so numerics match the DVE path
                        nc.scalar.activation(ta, xt[:], ACTF.Identity,
                                             bias=cvcol(CV_C128), scale=cvcol(CV_C15))
                        nc.vector.tensor_scalar(ta, ta, 143.0, 128.0,
                                                op0=ALU.min, op1=ALU.max)
                        intr = apad[i][:, 0, 1 + h0:1 + h0 + HH, 1:W + 1]
                        nc.vector.tensor_scalar(intr, ta, 128.0, None,
                                                op0=ALU.subtract)
                        # residual copy: keep x resident as bf16 (Pool)
                        nc.gpsimd.tensor_scalar(xres[i][:, h0:h0 + HH, :], xt[:],
                                                0.0, None, op0=ALU.add)

                # prep-ahead of 2 keeps each engine's in-order stream from
                # head-of-line-blocking the next image's prep behind this
                # image's PSUM copies
                phase1_prep(0)
                nc.gpsimd.dma_start(tw1[:], w1_in.ap())
                nc.gpsimd.dma_start(tw2[:], w2_in.ap())
                phase1_prep(1)
                for i in range(nper):
                    conv(i, tw1, s1p, ss1p)
                    if i + 2 < nper:
                        phase1_prep(i + 2)

            # ---------------- BN1 sync + coefs ----------------
            with nc.named_scope("bn1"):
                gst1 = stats_allreduce(s1p, ss1p, 1)
                uscale, ubias = _bn_coefs(nc, spool, gst1[:, 0:1], gst1[:, 1:2], cvcol, 1)

            # ---------------- phase2 (act_quant of BN1) + conv2 ----------------
            with nc.named_scope("conv2"):
                # first chunk covers rows 0..11 so group-0 matmuls (rows 0..9)
                # wait on one small chunk only
                P2CH = [(0, 12), (12, 16), (28, 16), (44, 12)]

                def phase2_prep(i):
                    for r0, rows in P2CH:
                        src = cint[i][:, r0:r0 + rows, :]
                        dsta = apad[i][:, 0, 1 + r0:1 + r0 + rows, 1:W + 1]
                        tr = trpool.tile([C, 16, W], BF16, tag="tr")
                        trv = tr[:, 0:rows, :]
                        nc.gpsimd.tensor_scalar(trv, src, uscale[:], ubias[:],
                                                op0=ALU.mult, op1=ALU.add)
                        nc.vector.tensor_scalar(trv, trv, 143.0, 128.0,
                                                op0=ALU.min, op1=ALU.max)
                        nc.vector.tensor_scalar(dsta, trv, 128.0, None,
                                                op0=ALU.subtract)
                    # build the shifted slab for the row-2 DoubleRow pair:
                    # slab1[r, 0:63] = slab0[r, 1:64] for the rows taps read
                    # (SBUF->SBUF DMA; the DMA engines are idle during conv2);
                    # two halves so the early groups' rows land sooner
                    if i >= 2:
                        nc.sync.dma_start(apad[i][:, 1, 2:34, 0:63],
                                          apad[i][:, 0, 2:34, 1:WP])
                        nc.sync.dma_start(apad[i][:, 1, 34:58, 0:63],
                                          apad[i][:, 0, 34:58, 1:WP])

                phase2_prep(0)
                phase2_prep(1)
                phase2_prep(2)
                for i in range(nper):
                    # images 0/1 keep the 3-single path: their shifted slab
                    # (gated by the BN1 coefs) isn't ready yet when PE is
                    conv(i, tw2, s2p, ss2p, pair_row2=(i >= 2),
                         sq_a_act=(i < nper - 1))
                    if i + 3 < nper:
                        phase2_prep(i + 3)

            # ---------------- BN2 sync + coefs ----------------
            with nc.named_scope("bn2"):
                gst2 = stats_allreduce(s2p, ss2p, 2)
                fscale, fbias = _bn_coefs(nc, spool, gst2[:, 0:1], gst2[:, 1:2], cvcol, 2)

            # ---------------- finalize: BN2 + residual, streamed stores ----------------
            with nc.named_scope("finalize"):
                CH = 14              # finalize chunk rows (divides H)
                NCH = H // CH
                for i in range(nper):
                    chunks = [(cidx * CH, CH) for cidx in range(NCH)]
                    if i == 0:
                        chunks = [(0, 7), (7, 7)] + chunks[1:]
                    for r0, rows in chunks:
                        src = cint[i][:, r0:r0 + rows, :]
                        t2f = fpool.tile([C, CH, W], F32, tag="fin", name="t2f")
                        t2 = t2f[:, 0:rows, :]
                        nc.scalar.activation(t2, src, ACTF.Identity,
                                             bias=fbias[:], scale=fscale[:])
                        # residual add in place; deep fpool keeps the store
                        # pipeline DMA-bound instead of buffer-reuse-bound
                        nc.vector.tensor_tensor(out=t2, in0=t2,
                                                in1=xres[i][:, r0:r0 + rows, :], op=ALU.add)
                        nc.sync.dma_start(out_d.ap()[i][:, r0:r0 + rows, :], t2)

            if rep_ctx is not None:
                rep_ctx.__exit__(None, None, None)

    nc.compile()
    return nc


_PROG_CACHE = {}


def _get_program(ncores, nper):
    key = (ncores, nper)
    if key not in _PROG_CACHE:
        _PROG_CACHE[key] = build_program(ncores, nper)
    return _PROG_CACHE[key]


def make_inputs(x, w1, w2, gamma1, beta1, gamma2, beta2, ncores=NCORES):
    """Host-side prep: shard x, quantize weights, build cvec."""
    x = np.asarray(x, dtype=np.float32)
    n = x.shape[0]
    nper = n // ncores
    assert nper * ncores == n

    def wq(w):
        w = np.asarray(w, dtype=np.float32)
        alpha = np.float32(np.abs(w).max()) + np.float32(1e-12)
        q = np.round(np.clip(w / alpha, -1.0, 1.0) * np.float32(7.0))
        return q.astype(np.float32), np.float32(alpha)

    q1, a1 = wq(w1)
    q2, a2 = wq(w2)
    # [co, ci, kh, kw] -> [ci, j, co], j ordered as DoubleRow pairs
    # [(0,dw),(1,dw)] for dw=0..2 then singles [(2,dw)]
    f8np = mybir.dt.np(F8)
    order = [(0, 0), (1, 0), (0, 1), (1, 1), (0, 2), (1, 2), (2, 0), (2, 1), (2, 2)]

    def pack(q):
        t = q.transpose(1, 2, 3, 0)
        return np.ascontiguousarray(
            np.stack([t[:, kh, kw, :] for kh, kw in order], axis=1)).astype(f8np)

    w1s = pack(q1)
    w2s = pack(q2)
    s1 = np.float32(a1 / np.float32(105.0))
    s2 = np.float32(a2 / np.float32(105.0))
    m = np.float32(n * H * W)

    g1 = np.asarray(gamma1, dtype=np.float32)
    g2 = np.asarray(gamma2, dtype=np.float32)
    b1 = np.asarray(beta1, dtype=np.float32)
    b2 = np.asarray(beta2, dtype=np.float32)

    cvec = np.zeros((C, CV_NCOLS), dtype=np.float32)
    cvec[:, CV_INVM] = np.float32(1.0) / m
    cvec[:, CV_K1] = s1 * s1 / m
    cvec[:, CV_K2] = s2 * s2 / m
    cvec[:, CV_SM1] = s1 / m
    cvec[:, CV_SM2] = s2 / m
    cvec[:, CV_CA1] = np.float32(15.0) * s1 * g1
    cvec[:, CV_CB1] = np.float32(15.0) * b1 + np.float32(128.0)
    cvec[:, CV_CA2] = s2 * g2
    cvec[:, CV_CB2] = b2
    cvec[:, CV_EPS] = 1e-5
    cvec[:, CV_C15] = 15.0
    cvec[:, CV_C128] = 128.0

    in_maps = []
    for c in range(ncores):
        in_maps.append({
            "x": np.ascontiguousarray(x[c * nper:(c + 1) * nper]),
            "w1s": w1s, "w2s": w2s, "cvec": cvec,
        })
    return in_maps, nper


def run(x, w1, w2, gamma1, beta1, gamma2, beta2, trace=False):
    in_maps, nper = make_inputs(x, w1, w2, gamma1, beta1, gamma2, beta2)
    nc = _get_program(NCORES, nper)
    res = bass_utils.run_bass_kernel_spmd(
        nc, in_maps, core_ids=list(range(NCORES)), trace=trace)
    out = np.concatenate([r["out"] for r in res.results], axis=0)
    return out, res


def kernel(x, w1, w2, gamma1, beta1, gamma2, beta2):
    out, _ = run(x, w1, w2, gamma1, beta1, gamma2, beta2)
    return out


# revision 60
# speedup vs baseline: 1.0486x; 1.0435x over previous
"""Trainium2 Bass kernel for a quantized ResNet BasicBlock.

Reference computation (per reference.py):
    out = act_quant(x); out = conv3x3(out, weight_quant(w1)); out = BN(out, g1, b1)
    out = act_quant(out); out = conv3x3(out, weight_quant(w2)); out = BN(out, g2, b2)
    return out + x
with act_quant(x) = round(clip(x,0,1)*15)/15 (4-bit), weight_quant symmetric 4-bit
per-tensor (levels -7..7, scale alpha/7, alpha = max|w|), BN in training mode
(batch stats over (N,H,W)).

Strategy (8 NeuronCores, data-parallel over batch, sync-BN via AllReduce):
  * Quantized activations are integers 0..15, weights integers -7..7 - both
    exact in fp8e4m3, and fp32 PSUM accumulation never rounds (sums < 2^24),
    so each conv3x3 is an EXACT integer computation.
  * Rounding is done by writing 15x+128 to bf16 (the [128,256) binade has
    step exactly 1, RNE matches jnp.round), then clip to [128,143] and
    subtract 128 into the unbiased fp8 activation image.
  * conv3x3 over a zero-padded [C=128 partitions, 59, 64] fp8 image
    (64-wide rows give the 16B-aligned row stride fp8 DoubleRow needs):
    per 8-row output group, 3 DoubleRow pair-matmuls contract taps
    (0,dw)+(1,dw) as K=256 in one pass (rhs is an overlapping [C,2,512]
    access pattern, pair stride = one row) plus 3 normal matmuls for the
    (2,dw) taps - 6 PE instructions instead of 9, streaming full 64-wide
    rows into a [C,8,64] PSUM bank (alignment columns are zero/ignored).
  * The PSUM->SBUF copy (ACT, accum_out) emits per-channel BN sums and
    stores conv results as int16 (|conv_int| ~ 2.3k); sum-of-squares via a
    DVE scalar_tensor_tensor with accum_out.  Per-channel sum/sumsq are
    AllReduced across the 8 cores ([128,2] fp32), then BN+act_quant collapse
    into one per-channel scale/bias applied to the integer conv output.
  * x is kept resident in SBUF as bf16 for the residual (no HBM reload;
    adds <=2^-9 relative rounding on the identity term, well inside the
    2e-2 gate), and the final BN2+residual is streamed per 28-row chunk
    straight to the output DMA so the post-AllReduce tail is store-bound.
"""

import os
import sys

for _p in ("/opt/trn_rl_repo", "/root/.axon_site/_ro/trn_rl_repo"):
    if os.path.isdir(_p) and _p not in sys.path:
        sys.path.insert(0, _p)

import numpy as np
import ml_dtypes

import concourse.bass as bass  # noqa: F401  (registers types)
import concourse.tile as tile
from concourse import bacc, mybir
from concourse import bass_utils

F32 = mybir.dt.float32
BF16 = mybir.dt.bfloat16
I16 = mybir.dt.int16
I32 = mybir.dt.int32
F8 = mybir.dt.float8e4
ACTF = mybir.ActivationFunctionType
ALU = mybir.AluOpType
AX = mybir.AxisListType

C = 128
H = W = 56
HP = 59               # padded rows (+1 spare zero row for stream overshoot)
WP = 64               # padded cols (16B-aligned rows for fp8 DoubleRow)
GR = 8                # output rows per PSUM group
NG = H // GR          # 7 groups per image
NCORES = 8

# cvec column indices (all [C] fp32, host-computed)
CV_INVM, CV_K1, CV_K2, CV_SM1, CV_SM2, CV_CA1, CV_CB1, CV_CA2, CV_CB2, \
    CV_EPS, CV_C15, CV_C128, CV_NCOLS = range(13)


def _bn_coefs(nc, pool, S, SS, cvcol, ph):
    """[C,1] coef math from global integer-unit sum S / sumsq SS to the fused
    scale/bias for this BN + following op.

    ph=1: (uscale, ubias): u = conv_int*uscale + ubias = 15*BN(y) + 128.
    ph=2: (fscale, fbias): out = conv_int*fscale + fbias = BN(y2).
    """
    idx = [0]

    def mk():
        idx[0] += 1
        return pool.tile([C, 1], F32, tag=f"bc{ph}_{idx[0]}", name=f"bc{ph}_{idx[0]}")

    kcol = CV_K1 if ph == 1 else CV_K2
    smcol = CV_SM1 if ph == 1 else CV_SM2
    acol = CV_CA1 if ph == 1 else CV_CA2
    bcol = CV_CB1 if ph == 1 else CV_CB2

    # critical path: ms -> ms2 -> vpe -> sqrt -> recip -> newton -> scale -> bias;
    # mean / sv / us0 hang off it in parallel (Pool).
    ms = mk()   # mean in real units, = S*(s/m)
    nc.vector.tensor_scalar(ms[:], S, cvcol(smcol), None, op0=ALU.mult)
    sv = mk()   # SS*s^2/m + eps  (on Pool, parallel)
    nc.gpsimd.tensor_scalar(sv[:], SS, cvcol(kcol), cvcol(CV_EPS),
                            op0=ALU.mult, op1=ALU.add)
    mean = mk()  # mean in integer units (only needed for the bias at the end)
    nc.gpsimd.tensor_scalar(mean[:], S, cvcol(CV_INVM), None, op0=ALU.mult)
    ms2 = mk()
    nc.vector.tensor_tensor(out=ms2[:], in0=ms[:], in1=ms[:], op=ALU.mult)
    vpe = mk()  # var_real + eps
    nc.vector.tensor_tensor(out=vpe[:], in0=sv[:], in1=ms2[:], op=ALU.subtract)
    # rsqrt without the scalar engine (avoids Sqrt<->Identity act-table
    # reloads): fast-inverse-sqrt seed j = MAGIC - (bits(v) >> 1) computed as
    # MAGIC - 0.5*bits(v) in the fp32 datapath (<=64-ulp bit error on a 3.4%
    # seed), then two Newton steps r <- r*(1.5 - 0.5*v*r^2) for ~5e-6 rel.
    seed = pool.tile([C, 1], I32, tag=f"bseed{ph}", name=f"bseed{ph}")
    nc.vector.tensor_scalar(seed[:], vpe[:].bitcast(I32), -0.5, 1597463007.0,
                            op0=ALU.mult, op1=ALU.add)
    r0 = seed[:].bitcast(F32)
    t1 = mk()
    nc.vector.tensor_tensor(out=t1[:], in0=r0, in1=r0, op=ALU.mult)
    nc.vector.tensor_tensor(out=t1[:], in0=t1[:], in1=vpe[:], op=ALU.mult)
    nc.vector.tensor_scalar(t1[:], t1[:], -0.5, 1.5, op0=ALU.mult, op1=ALU.add)
    r1 = mk()
    nc.vector.tensor_tensor(out=r1[:], in0=r0, in1=t1[:], op=ALU.mult)
    # us0 = CA*r1 on Pool, parallel with the second Newton chain
    us0 = mk()
    nc.gpsimd.tensor_tensor(out=us0[:], in0=cvcol(acol), in1=r1[:], op=ALU.mult)
    t2 = mk()
    nc.vector.tensor_tensor(out=t2[:], in0=r1[:], in1=r1[:], op=ALU.mult)
    nc.vector.tensor_tensor(out=t2[:], in0=t2[:], in1=vpe[:], op=ALU.mult)
    nc.vector.tensor_scalar(t2[:], t2[:], -0.5, 1.5, op0=ALU.mult, op1=ALU.add)
    scale = mk()
    nc.vector.tensor_tensor(out=scale[:], in0=us0[:], in1=t2[:], op=ALU.mult)
    mb = mk()
    nc.vector.tensor_tensor(out=mb[:], in0=scale[:], in1=mean[:], op=ALU.mult)
    bias = mk()
    nc.vector.scalar_tensor_tensor(out=bias[:], in0=mb[:], scalar=-1.0,
                                   in1=cvcol(bcol), op0=ALU.mult, op1=ALU.add)
    return scale, bias


def build_program(ncores, nper, collective=True, reps=1):
    nc = bacc.Bacc("TRN2", target_bir_lowering=False, debug=False, num_devices=ncores)

    x_in = nc.dram_tensor("x", [nper, C, H, W], F32, kind="ExternalInput")
    w1_in = nc.dram_tensor("w1s", [C, 9, C], F8, kind="ExternalInput")
    w2_in = nc.dram_tensor("w2s", [C, 9, C], F8, kind="ExternalInput")
    cv_in = nc.dram_tensor("cvec", [C, CV_NCOLS], F32, kind="ExternalInput")
    out_d = nc.dram_tensor("out", [nper, C, H, W], BF16, kind="ExternalOutput")

    with tile.TileContext(nc) as tc:
        with tc.tile_pool(name="const", bufs=1) as cpool, \
             tc.tile_pool(name="apad", bufs=nper) as apool, \
             tc.tile_pool(name="cint", bufs=nper) as ipool, \
             tc.tile_pool(name="xres", bufs=nper) as xrespool, \
             tc.tile_pool(name="xin", bufs=3) as xpool, \
             tc.tile_pool(name="tr", bufs=3) as trpool, \
             tc.tile_pool(name="sq", bufs=2) as sqpool, \
             tc.tile_pool(name="fin", bufs=5) as fpool, \
             tc.tile_pool(name="stat", bufs=1) as spool, \
             tc.tile_pool(name="psum", bufs=1, space="PSUM") as ppool, \
             tc.tile_pool(name="dram", bufs=1, space="DRAM") as dpool:

            tw1 = cpool.tile([C, 9, C], F8, tag="w1")
            tw2 = cpool.tile([C, 9, C], F8, tag="w2")
            tcv = cpool.tile([C, CV_NCOLS], F32, tag="cv")
            # cvec on SP first (x loads queue right behind it); weight DMAs
            # are emitted after the first image's prep, via Pool's SWDGE, so
            # the first x chunk wins the DMA device
            nc.sync.dma_start(tcv[:], cv_in.ap())

            def cvcol(j):
                return tcv[:, j:j + 1]

            # dep-free Identity op at t=0 soaks up the one-time act-table load
            warm = cpool.tile([C, 1], F32, tag="warm")
            nc.scalar.activation(warm[:], warm[:], ACTF.Identity, bias=0.0, scale=1.0)

            # slab 0 = padded activation image; slab 1 = the same image
            # shifted left one column (built by an SBUF->SBUF DMA before
            # conv2) so taps (2,0)+(2,1) pair as one DoubleRow matmul with
            # pair stride HP*WP (16B-aligned)
            apad = [apool.tile([C, 2, HP, WP], F8, tag="apad", name=f"apad{i}") for i in range(nper)]
            cint = [ipool.tile([C, H, W], I16, tag="cint", name=f"cint{i}") for i in range(nper)]
            xres = [xrespool.tile([C, H, W], BF16, tag="xres", name=f"xres{i}") for i in range(nper)]

            rep_ctx = tc.For_i(0, reps, 1) if reps > 1 else None
            if rep_ctx is not None:
                rep_ctx.__enter__()

            GA = 4                               # groups in PSUM tile A
            GB = NG - GA                         # groups in PSUM tile B
            npart = nper * 2 + 1                 # +1: last image's split B
            s1p = spool.tile([C, npart], F32, tag="s1p")
            ss1p = spool.tile([C, npart], F32, tag="ss1p")
            s2p = spool.tile([C, npart], F32, tag="s1p", name="s2p")
            ss2p = spool.tile([C, npart], F32, tag="ss1p", name="ss2p")
            psA = ppool.tile([C, GA * GR, WP], F32, tag="psA", bufs=1)
            psB = ppool.tile([C, GB * GR, WP], F32, tag="psB", bufs=1)

            def conv(i, tw, sp, ssp, pair_row2=False, sq_a_act=False):
                """conv3x3 of apad[i], group-major: per 8-row output group,
                3 DoubleRow pair-matmuls (taps (0,dw)+(1,dw), K=256) + the
                dh=2 taps (3 singles, or with pair_row2 one cross-slab DR
                for (2,0)+(2,1) plus one single) into one PSUM bank; groups
                0-3 fill the 4-bank tile A, groups 4-6 the 3-bank tile B,
                and each tile's single PSUM->SBUF copy (ACT, accum_out=BN
                sum) drains while the other tile's matmuls stream."""
                flat = apad[i].rearrange("c s h w -> c (s h w)")
                NFLAT = GR * WP

                def emit_group(g, psview):
                    out = psview.rearrange("c a b -> c (a b)")
                    for p in range(3):           # DR pairs, dw = p
                        base = g * GR * WP + p
                        rhs = flat[:, base:base + NFLAT]
                        rhs.ap.insert(1, [WP, 2])        # [C, 2, 512]
                        nc.tensor.matmul(out, tw[:, 2 * p:2 * p + 2, :], rhs,
                                         start=(p == 0), stop=False,
                                         perf_mode=mybir.MatmulPerfMode.DoubleRow)
                    if pair_row2:
                        # (2,0) in slab 0 pairs with (2,1) = slab 1 @ (2,0)
                        base = (g * GR + 2) * WP
                        rhs = flat[:, base:base + NFLAT]
                        rhs.ap.insert(1, [HP * WP, 2])   # cross-slab pair
                        nc.tensor.matmul(out, tw[:, 6:8, :], rhs,
                                         start=False, stop=False,
                                         perf_mode=mybir.MatmulPerfMode.DoubleRow)
                        base = (g * GR + 2) * WP + 2
                        rhs = flat[:, base:base + NFLAT]
                        nc.tensor.matmul(out, tw[:, 8, :], rhs,
                                         start=False, stop=True)
                    else:
                        for s in range(3):       # singles, dh = 2, dw = s
                            base = (g * GR + 2) * WP + s
                            rhs = flat[:, base:base + NFLAT]
                            nc.tensor.matmul(out, tw[:, 6 + s, :], rhs,
                                             start=False, stop=(s == 2))

                def copy_chunk(k, r0, pt, sq_act=False):
                    rows = pt.shape[1]
                    dst = cint[i][:, r0:r0 + rows, :]
                    nc.scalar.activation(dst, pt[:, :, 0:W], ACTF.Identity,
                                         bias=0.0, scale=1.0,
                                         accum_out=sp[:, k:k + 1])
                    sq = sqpool.tile([C, GA * GR, W], BF16, tag="sq")
                    if sq_act:
                        # ACT Square (1-ULP family, same act-table set as
                        # Identity) frees DVE in the paired conv2 regime
                        nc.scalar.activation(sq[:, 0:rows, :], dst, ACTF.Square,
                                             bias=0.0, scale=1.0,
                                             accum_out=ssp[:, k:k + 1])
                    else:
                        nc.vector.scalar_tensor_tensor(
                            out=sq[:, 0:rows, :], in0=dst, scalar=1.0, in1=dst,
                            op0=ALU.mult, op1=ALU.mult,
                            accum_out=ssp[:, k:k + 1])

                for g in range(GA):
                    emit_group(g, psA[:, g * GR:(g + 1) * GR, :])
                copy_chunk(2 * i, 0, psA[:], sq_act=sq_a_act)
                if i == nper - 1:
                    # last image: drain PSUM B in two pieces so the final
                    # copy+sumsq feeding the BN stats reduce is small
                    emit_group(GA, psB[:, 0:GR, :])
                    emit_group(GA + 1, psB[:, GR:2 * GR, :])
                    copy_chunk(2 * i + 1, GA * GR, psB[:, 0:2 * GR, :])
                    emit_group(GA + 2, psB[:, 2 * GR:3 * GR, :])
                    copy_chunk(2 * nper, (GA + 2) * GR, psB[:, 2 * GR:3 * GR, :])
                else:
                    for g in range(GB):
                        emit_group(GA + g, psB[:, g * GR:(g + 1) * GR, :])
                    copy_chunk(2 * i + 1, GA * GR, psB[:])

            def stats_allreduce(sp, ssp, tag):
                st = spool.tile([C, 2], F32, tag=f"st{tag}")
                nc.vector.tensor_reduce(out=st[:, 0:1], in_=sp[:], axis=AX.X, op=ALU.add)
                nc.vector.tensor_reduce(out=st[:, 1:2], in_=ssp[:], axis=AX.X, op=ALU.add)
                if not collective:
                    return st
                din = dpool.tile([C, 2], F32, tag=f"din{tag}")
                dout = dpool.tile([C, 2], F32, tag=f"dout{tag}")
                nc.gpsimd.dma_start(din[:], st[:])
                nc.gpsimd.collective_compute(
                    "AllReduce", ALU.add,
                    replica_groups=[list(range(ncores))],
                    ins=[din.opt()], outs=[dout.opt()])
                gst = spool.tile([C, 2], F32, tag=f"gst{tag}")
                nc.gpsimd.dma_start(gst[:], dout[:])
                return gst

            # ---------------- stage A + conv1 ----------------
            with nc.named_scope("conv1"):
                HH = H // 4

                def phase1_prep(i):
                    # zero the conv border (incl. alignment cols); on DVE, so
                    # the dep-free memsets never crowd out the clamps
                    nc.vector.memset(apad[i][:, 0, 0, :], 0)
                    nc.vector.memset(apad[i][:, 0, 57:HP, :], 0)
                    nc.vector.memset(apad[i][:, 0, 1:57, 0:1], 0)
                    nc.vector.memset(apad[i][:, 0, 1:57, 57:WP], 0)
                    # shifted slab's last column never receives DMA data
                    nc.vector.memset(apad[i][:, 1, 2:58, 63:WP], 0)
                    for h0 in range(0, H, HH):
                        xt = xpool.tile([C, HH, W], F32, tag="x")
                        nc.sync.dma_start(xt[:], x_in.ap()[i][:, h0:h0 + HH, :])
                        tat = trpool.tile([C, 16, W], BF16, tag="tr", name="tat")
                        ta = tat[:, 0:HH, :]
                        # u = 15x+128 -> bf16 write rounds to integer grid
                        # (RNE); same ACT Identity scale/bias pattern phase2
                        # uses, so numerics match the DVE path
                        nc.scalar.activation(ta, xt[:], ACTF.Identity,
                                             bias=cvcol(CV_C128), scale=cvcol(CV_C15))
                        nc.vector.tensor_scalar(ta, ta, 143.0, 128.0,
                                                op0=ALU.min, op1=ALU.max)
                        intr = apad[i][:, 0, 1 + h0:1 + h0 + HH, 1:W + 1]
                        nc.vector.tensor_scalar(intr, ta, 128.0, None,
                                                op0=ALU.subtract)
                        # residual copy: keep x resident as bf16 (Pool)
                        nc.gpsimd.tensor_scalar(xres[i][:, h0:h0 + HH, :], xt[:],
                                                0.0, None, op0=ALU.add)

                # prep-ahead of 2 keeps each engine's in-order stream from
                # head-of-line-blocking the next image's prep behind this
                # image's PSUM copies
                phase1_prep(0)
                nc.gpsimd.dma_start(tw1[:], w1_in.ap())
                nc.gpsimd.dma_start(tw2[:], w2_in.ap())
                phase1_prep(1)
                for i in range(nper):
                    conv(i, tw1, s1p, ss1p)
                    if i + 2 < nper:
                        phase1_prep(i + 2)

            # ---------------- BN1 sync + coefs ----------------
            with nc.named_scope("bn1"):
                gst1 = stats_allreduce(s1p, ss1p, 1)
                uscale, ubias = _bn_coefs(nc, spool, gst1[:, 0:1], gst1[:, 1:2], cvcol, 1)

            # ---------------- phase2 (act_quant of BN1) + conv2 ----------------
            with nc.named_scope("conv2"):
                # first chunk covers rows 0..11 so group-0 matmuls (rows 0..9)
                # wait on one small chunk only
                P2CH = [(0, 12), (12, 16), (28, 16), (44, 12)]

                def phase2_prep(i):
                    for r0, rows in P2CH:
                        src = cint[i][:, r0:r0 + rows, :]
                        dsta = apad[i][:, 0, 1 + r0:1 + r0 + rows, 1:W + 1]
                        tr = trpool.tile([C, 16, W], BF16, tag="tr")
                        trv = tr[:, 0:rows, :]
                        nc.gpsimd.tensor_scalar(trv, src, uscale[:], ubias[:],
                                                op0=ALU.mult, op1=ALU.add)
                        nc.vector.tensor_scalar(trv, trv, 143.0, 128.0,
                                                op0=ALU.min, op1=ALU.max)
                        nc.vector.tensor_scalar(dsta, trv, 128.0, None,
                                                op0=ALU.subtract)
                    # build the shifted slab for the row-2 DoubleRow pair:
                    # slab1[r, 0:63] = slab0[r, 1:64] for the rows taps read
                    # (SBUF->SBUF DMA; the DMA engines are idle during conv2);
                    # two halves so the early groups' rows land sooner
                    if i >= 2:
                        nc.sync.dma_start(apad[i][:, 1, 2:34, 0:63],
                                          apad[i][:, 0, 2:34, 1:WP])
                        nc.sync.dma_start(apad[i][:, 1, 34:58, 0:63],
                                          apad[i][:, 0, 34:58, 1:WP])

                phase2_prep(0)
                phase2_prep(1)
                phase2_prep(2)
                for i in range(nper):
                    # images 0/1 keep the 3-single path: their shifted slab
                    # (gated by the BN1 coefs) isn't ready yet when PE is
                    conv(i, tw2, s2p, ss2p, pair_row2=(i >= 2),
                         sq_a_act=(i < nper - 1))
                    if i + 3 < nper:
                        phase2_prep(i + 3)

            # ---------------- BN2 sync + coefs ----------------
            with nc.named_scope("bn2"):
                gst2 = stats_allreduce(s2p, ss2p, 2)
                fscale, fbias = _bn_coefs(nc, spool, gst2[:, 0:1], gst2[:, 1:2], cvcol, 2)

            # ---------------- finalize: BN2 + residual, streamed stores ----------------
            with nc.named_scope("finalize"):
                CH = 14              # finalize chunk rows (divides H)
                NCH = H // CH
                for i in range(nper):
                    chunks = [(cidx * CH, CH) for cidx in range(NCH)]
                    if i == 0:
                        chunks = [(0, 7), (7, 7)] + chunks[1:]
                    for r0, rows in chunks:
                        src = cint[i][:, r0:r0 + rows, :]
                        t2f = fpool.tile([C, CH, W], F32, tag="fin", name="t2f")
                        t2 = t2f[:, 0:rows, :]
                        nc.scalar.activation(t2, src, ACTF.Identity,
                                             bias=fbias[:], scale=fscale[:])
                        # residual add rounds once into a bf16 store tile:
                        # halves the post-AllReduce store traffic (bounded
                        # +2e-3 rel on the 2e-2 gate); adds alternate
                        # DVE/Pool so neither engine paces the tail
                        obf = fpool.tile([C, CH, W], BF16, tag="obf", name="obf")
                        ob = obf[:, 0:rows, :]
                        aeng = nc.vector if (i * NCH) % 2 == 0 else nc.gpsimd
                        aeng.tensor_tensor(out=ob, in0=t2,
                                           in1=xres[i][:, r0:r0 + rows, :], op=ALU.add)
                        nc.sync.dma_start(out_d.ap()[i][:, r0:r0 + rows, :], ob)

            if rep_ctx is not None:
                rep_ctx.__exit__(None, None, None)

    nc.compile()
    return nc


_PROG_CACHE = {}


def _get_program(ncores, nper):
    key = (ncores, nper)
    if key not in _PROG_CACHE:
        _PROG_CACHE[key] = build_program(ncores, nper)
    return _PROG_CACHE[key]


def make_inputs(x, w1, w2, gamma1, beta1, gamma2, beta2, ncores=NCORES):
    """Host-side prep: shard x, quantize weights, build cvec."""
    x = np.asarray(x, dtype=np.float32)
    n = x.shape[0]
    nper = n // ncores
    assert nper * ncores == n

    def wq(w):
        w = np.asarray(w, dtype=np.float32)
        alpha = np.float32(np.abs(w).max()) + np.float32(1e-12)
        q = np.round(np.clip(w / alpha, -1.0, 1.0) * np.float32(7.0))
        return q.astype(np.float32), np.float32(alpha)

    q1, a1 = wq(w1)
    q2, a2 = wq(w2)
    # [co, ci, kh, kw] -> [ci, j, co], j ordered as DoubleRow pairs
    # [(0,dw),(1,dw)] for dw=0..2 then singles [(2,dw)]
    f8np = mybir.dt.np(F8)
    order = [(0, 0), (1, 0), (0, 1), (1, 1), (0, 2), (1, 2), (2, 0), (2, 1), (2, 2)]

    def pack(q):
        t = q.transpose(1, 2, 3, 0)
        return np.ascontiguousarray(
            np.stack([t[:, kh, kw, :] for kh, kw in order], axis=1)).astype(f8np)

    w1s = pack(q1)
    w2s = pack(q2)
    s1 = np.float32(a1 / np.float32(105.0))
    s2 = np.float32(a2 / np.float32(105.0))
    m = np.float32(n * H * W)

    g1 = np.asarray(gamma1, dtype=np.float32)
    g2 = np.asarray(gamma2, dtype=np.float32)
    b1 = np.asarray(beta1, dtype=np.float32)
    b2 = np.asarray(beta2, dtype=np.float32)

    cvec = np.zeros((C, CV_NCOLS), dtype=np.float32)
    cvec[:, CV_INVM] = np.float32(1.0) / m
    cvec[:, CV_K1] = s1 * s1 / m
    cvec[:, CV_K2] = s2 * s2 / m
    cvec[:, CV_SM1] = s1 / m
    cvec[:, CV_SM2] = s2 / m
    cvec[:, CV_CA1] = np.float32(15.0) * s1 * g1
    cvec[:, CV_CB1] = np.float32(15.0) * b1 + np.float32(128.0)
    cvec[:, CV_CA2] = s2 * g2
    cvec[:, CV_CB2] = b2
    cvec[:, CV_EPS] = 1e-5
    cvec[:, CV_C15] = 15.0
    cvec[:, CV_C128] = 128.0

    in_maps = []
    for c in range(ncores):
        in_maps.append({
            "x": np.ascontiguousarray(x[c * nper:(c + 1) * nper]),
            "w1s": w1s, "w2s": w2s, "cvec": cvec,
        })
    return in_maps, nper


def run(x, w1, w2, gamma1, beta1, gamma2, beta2, trace=False):
    in_maps, nper = make_inputs(x, w1, w2, gamma1, beta1, gamma2, beta2)
    nc = _get_program(NCORES, nper)
    res = bass_utils.run_bass_kernel_spmd(
        nc, in_maps, core_ids=list(range(NCORES)), trace=trace)
    out = np.concatenate([r["out"] for r in res.results], axis=0).astype(np.float32)
    return out, res


def kernel(x, w1, w2, gamma1, beta1, gamma2, beta2):
    out, _ = run(x, w1, w2, gamma1, beta1, gamma2, beta2)
    return out
